# revision 81
# baseline (speedup 1.0000x reference)
"""Causal self-attention (GQA, RoPE) Trainium2 Bass kernel.

Full inputs in, full output out. Tensor-parallel over heads across 8
NeuronCores: core i computes q-heads 4i..4i+3 (kv head i) and a partial
output projection over its 256 attn-out features; the host sums the 8
partial outputs (the "all-reduce after output_proj" step).

v2 layout: scores are computed pre-transposed (S^T = k^T q, with k on
the psum partition dim), so the attention-probability transposes of the
v1 kernel disappear; the softmax denominator rides along the AV matmul
as an extra ones column of V, and the per-query normalization uses a
rank-1 (K=1) broadcast matmul plus one vector multiply. x arrives
host-pre-transposed, killing the on-chip x transposes.
"""

import numpy as np

import concourse.bacc as bacc
import concourse.mybir as mybir
import concourse.tile as tile
from concourse.bass_utils import run_bass_kernel_spmd

S = 2048          # sequence length
E = 2048          # embedding dim
H = 32            # query heads
KV = 8            # kv heads
HD = 64           # head dim
NCORES = 8
HC = H // NCORES  # query heads per core = 4
DQ = HC * HD      # per-core q proj width = 256
DKV = HD          # per-core kv proj width = 64
DQK = DQ + DKV    # roped span = 320
DW = DQ + 2 * DKV  # fused qkv proj width = 384
ST = S // 128     # 16 s-tiles of 128 rows
MASK_NEG = -1.0e4  # pre-scale additive mask (scaled: -1250 -> exp == 0)

F32 = mybir.dt.float32
F32R = mybir.dt.float32r
BF16 = mybir.dt.bfloat16
NP_BF16 = mybir.dt.np(BF16)


def r(ap):
    """Bitcast an AP to float32r so the PE runs fast-mode fp32 matmuls."""
    return ap.bitcast(F32R)


def build_nc(seq_tiles=ST, reps=1, phases=(1, 2, 3)):
    """Build + compile the per-core Bass program (identical on all cores)."""
    st_n = seq_tiles
    s_n = st_n * 128
    ng = st_n // 4    # 512-wide q groups

    nc = bacc.Bacc("TRN2", target_bir_lowering=False, debug=False)
    xt_d = nc.dram_tensor("xt", [E, s_n], BF16, kind="ExternalInput")
    wt_d = nc.dram_tensor("wt", [E, DW], BF16, kind="ExternalInput")
    wot_d = nc.dram_tensor("wot", [DQ, E], F32R, kind="ExternalInput")
    cs_d = nc.dram_tensor("cs", [s_n, 2 * (DQK // 2)], F32, kind="ExternalInput")
    mask_d = nc.dram_tensor("maskt", [128, 4 * 512], F32, kind="ExternalInput")
    id_d = nc.dram_tensor("ident", [128, 128], F32R, kind="ExternalInput")
    idb_d = nc.dram_tensor("identb", [128, 128], BF16, kind="ExternalInput")
    one_d = nc.dram_tensor("ones", [128, 64], F32R, kind="ExternalInput")
    oneb_d = nc.dram_tensor("onesb", [128, 4], BF16, kind="ExternalInput")
    out_d = nc.dram_tensor("out", [s_n, E], BF16, kind="ExternalOutput")

    with tile.TileContext(nc) as tc:
        with (
            tc.tile_pool(name="const", bufs=1) as constp,
            tc.tile_pool(name="store", bufs=1) as storep,
            tc.tile_pool(name="store2", bufs=2) as storep2,
            tc.tile_pool(name="p1x", bufs=2) as p1x,
            tc.tile_pool(name="p1q", bufs=2) as p1q,
            tc.tile_pool(name="pexp", bufs=4) as pexp,
            tc.tile_pool(name="prv", bufs=2) as prv,
            tc.tile_pool(name="pbc", bufs=2) as pbc,
            tc.tile_pool(name="po", bufs=2) as po,
            # PSUM: psA 2x1 + psS 2x2 + psV 2x1 = 8 banks exactly
            tc.tile_pool(name="psA", bufs=2, space="PSUM") as psA,
            tc.tile_pool(name="psS", bufs=2, space="PSUM") as psS,
            tc.tile_pool(name="psV", bufs=2, space="PSUM") as psV,
        ):
            # one-time const loads, spread across idle queues so SP can
            # start on the first x chunk immediately (SP serializes DMAs).
            ident = constp.tile([128, 128], F32R)
            nc.scalar.dma_start(out=ident[:], in_=id_d.ap()[:, :])
            identb = constp.tile([128, 128], BF16)
            nc.scalar.dma_start(out=identb[:], in_=idb_d.ap()[:, :])
            ones_sb = constp.tile([128, 64], F32R)
            nc.scalar.dma_start(out=ones_sb[:], in_=one_d.ap()[:, :])
            onesb_sb = constp.tile([128, 4], BF16)
            nc.scalar.dma_start(out=onesb_sb[:], in_=oneb_d.ap()[:, :])
            wT_sb = constp.tile([128, E // 128, DW], BF16)
            wtv = wt_d.ap().rearrange("(j p) w -> p j w", p=128)
            nc.scalar.dma_start(out=wT_sb[:, 0:8, :], in_=wtv[:, 0:8, :])
            nc.gpsimd.dma_start(out=wT_sb[:, 8:16, :], in_=wtv[:, 8:16, :])
            # whole RoPE cos/sin table resident in SBUF (needed early)
            cst = constp.tile([128, st_n, 2, DQK // 2], F32)
            nc.gpsimd.dma_start(
                out=cst[:],
                in_=cs_d.ap().rearrange("(t p) (a b) -> p t a b", p=128, a=2),
            )
            mask_sb = constp.tile([128, 4, 512], F32)
            nc.gpsimd.dma_start(
                out=mask_sb[:], in_=mask_d.ap().rearrange("p (j k) -> p j k", j=4)
            )
            woT_sb = constp.tile([128, 2, E], F32R)
            nc.gpsimd.dma_start(
                out=woT_sb[:], in_=wot_d.ap().rearrange("(c p) e -> p c e", p=128)
            )

            for _rep in range(reps):
                # persistent per-group stores. kT/vo live until the last
                # attention group reads them, so they are double-buffered
                # (storep2) to let the next rep's phase 1 overlap this
                # rep's tail.
                qT = []   # [64, 4 heads x 512]: head h s-tile tt at h*512+tt*128
                kT = []   # [64, 512]
                vo = []   # [128, 4, 65]: per chunk [v(64) | 1]
                ao = []   # [128, 2, 512]: attn-out^T, head pair c on dim 1
                for g in range(ng):
                    qT.append(storep.tile([64, HC * 512], BF16, tag=f"qT{g}",
                                          name=f"qT{g}"))
                    kT.append(storep2.tile([64, 512], BF16, tag=f"kT{g}",
                                           name=f"kT{g}"))
                    vo.append(storep2.tile([128, 4, 65], BF16, tag=f"vo{g}",
                                           name=f"vo{g}"))
                    ao.append(storep.tile([128, 2, 512], F32R, tag=f"ao{g}",
                                          name=f"ao{g}"))
                for g in range(ng):
                    nc.vector.tensor_copy(
                        vo[g][:, :, 64:65],
                        onesb_sb[:].rearrange("p (a b) -> p a b", b=1),
                    )

                # ---------------- phase 1: qkv proj + rope ----------------
                def load_x_half(g, half):
                    c0 = g * 512 + half * 256
                    xts = p1x.tile([128, E // 128, 256], BF16, tag="xts")
                    for q in range(2):  # split so the first s-tile lands early
                        nc.sync.dma_start(
                            out=xts[:, :, q * 128:(q + 1) * 128],
                            in_=xt_d.ap()[:, c0 + q * 128:c0 + (q + 1) * 128]
                            .rearrange("(j p) c -> p j c", p=128),
                        )
                    return xts

                def phase1_tile(t, xts):
                    """Generator: yields between instruction clumps so the
                    emitter can interleave phase-1 PE work into the
                    Act-paced attention chunk stream of the previous group."""
                    g, tt = divmod(t, 4)
                    ps_qkv = psA.tile([128, 512], F32, tag="a")
                    h0 = (tt % 2) * 128
                    for j in range(E // 128):
                        nc.tensor.matmul(
                            ps_qkv[:, 0:DW],
                            xts[:, j, h0:h0 + 128],
                            wT_sb[:, j, :],
                            start=(j == 0),
                            stop=(j == E // 128 - 1),
                        )
                        if j % 4 == 3:
                            yield
                    # rope on q+k jointly (320 cols)
                    pairs = DQK // 2
                    qk = p1q.tile([128, DQK], BF16, tag="qk")
                    se = ps_qkv[:, 0:DQK].rearrange("p (n two) -> p two n", two=2)
                    de = qk[:].rearrange("p (n two) -> p two n", two=2)
                    c_ap = cst[:, t, 0, :]
                    s_ap = cst[:, t, 1, :]
                    t1 = p1q.tile([128, pairs], F32, tag="t1")
                    t2 = p1q.tile([128, pairs], F32, tag="t2")
                    nc.vector.tensor_mul(t1[:], se[:, 0, :], c_ap)
                    nc.vector.tensor_mul(t2[:], se[:, 1, :], s_ap)
                    nc.vector.tensor_sub(de[:, 0, :], t1[:], t2[:])
                    yield
                    t3 = p1q.tile([128, pairs], F32, tag="t3")
                    t4 = p1q.tile([128, pairs], F32, tag="t4")
                    nc.vector.tensor_mul(t3[:], se[:, 1, :], c_ap)
                    nc.vector.tensor_mul(t4[:], se[:, 0, :], s_ap)
                    nc.vector.tensor_add(de[:, 1, :], t3[:], t4[:])
                    # v chunk (cols 0:64 of the 65-col [v|1] slot)
                    nc.scalar.copy(vo[g][:, tt, 0:64], ps_qkv[:, DQK:DW])
                    yield
                    # transposes: q heads 0,1 + k, then q heads 2,3
                    tr1 = psS.tile([128, 1024], F32, tag="s")
                    t1b = tr1[:].bitcast(BF16)
                    nc.tensor.matmul(t1b[0:64, 0:128], qk[:, 0:64],
                                     identb[:], is_transpose=True,
                                     start=True, stop=False)
                    nc.tensor.matmul(t1b[0:64, 128:256], qk[:, 64:128],
                                     identb[:], is_transpose=True,
                                     start=False, stop=False)
                    nc.tensor.matmul(t1b[0:64, 256:384], qk[:, 256:320],
                                     identb[:], is_transpose=True,
                                     start=False, stop=True)
                    qTv = qT[g][:].rearrange("p (h c) -> p h c", c=512)
                    nc.scalar.copy(
                        qTv[:, 0:2, tt * 128:(tt + 1) * 128],
                        t1b[0:64, 0:256].rearrange("p (h c) -> p h c", c=128),
                    )
                    nc.vector.tensor_copy(
                        kT[g][:, tt * 128:(tt + 1) * 128], t1b[0:64, 256:384]
                    )
                    yield
                    tr2 = psS.tile([128, 1024], F32, tag="s")
                    t2b = tr2[:].bitcast(BF16)
                    nc.tensor.matmul(t2b[0:64, 0:128], qk[:, 128:192],
                                     identb[:], is_transpose=True,
                                     start=True, stop=False)
                    nc.tensor.matmul(t2b[0:64, 128:256], qk[:, 192:256],
                                     identb[:], is_transpose=True,
                                     start=False, stop=True)
                    nc.vector.tensor_copy(
                        qTv[:, 2:4, tt * 128:(tt + 1) * 128],
                        t2b[0:64, 0:256].rearrange("p (h c) -> p h c", c=128),
                    )
                    yield

                def phase1_group(g):
                    """Generator over all of group g's phase-1 micro-steps."""
                    for half in range(2):
                        xts = load_x_half(g, half)
                        yield
                        for tt in (2 * half, 2 * half + 1):
                            yield from phase1_tile(4 * g + tt, xts)

                # ---------------- phase 2: attention (transposed) ----------
                def phase2_group(g, fill=None, lead=0):
                    """fill: callable pulling one fill micro-step (next
                    group's phase 1, earlier groups' phase 3), interleaved
                    between attention chunk pairs to keep PE fed while the
                    Act engine paces the exp stream. Chunks are processed in
                    PAIRS sharing one 2-bank psum tile so non-diagonal pairs
                    need a single exp; AV matmuls trail one pair behind the
                    scores, and each head's normalization is deferred into
                    the next head's stream (hides recip->bcast latency)."""
                    def pull(k):
                        for _ in range(k):
                            if fill is None or not fill():
                                return

                    def make_norm(ps_av, p0, hp2):
                        def emit():
                            # rows 0:64 = head out^T, row 64 = softmax denom.
                            # Broadcast 1/denom down 64 rows via K=1 matmul.
                            rv = prv.tile([128, 512], F32R, tag="rv")
                            with nc.allow_low_precision(reason="f32r==f32"):
                                nc.vector.reciprocal(
                                    rv[64:65, :], ps_av[64:65, :]
                                )
                            ps_b = psS.tile([128, 1024], F32, tag="s")
                            nc.tensor.matmul(
                                ps_b[0:64, 0:512], ones_sb[64:65, 0:64],
                                rv[64:65, :], start=True, stop=True,
                            )
                            bc = pbc.tile([128, 512], F32, tag="bc")
                            nc.vector.tensor_copy(bc[p0:p0 + 64, :],
                                                  ps_b[0:64, 0:512])
                            nc.vector.tensor_mul(
                                ao[g][p0:p0 + 64, hp2, :],
                                ps_av[0:64, :],
                                bc[p0:p0 + 64, :],
                            )
                        return emit

                    ci = 0
                    pend_norm = None
                    nch = 4 * g + 4
                    for h in range(HC):
                        p0 = 64 * (h & 1)
                        hp2 = h >> 1
                        ps_av = psV.tile([128, 512], F32, tag="v")
                        pend_av = []
                        for pk in range(nch // 2):
                            kcs = (2 * pk, 2 * pk + 1)
                            ps_s = psS.tile([128, 1024], F32, tag="s")
                            los = []
                            for idx, kc in enumerate(kcs):
                                kg, ko = divmod(kc, 4)
                                j = kc - 4 * g
                                lo = 0 if j < 0 else 128 * j
                                los.append(lo)
                                base = idx * 512
                                nc.tensor.matmul(
                                    ps_s[:, base + lo:base + 512],
                                    kT[kg][:, ko * 128:(ko + 1) * 128],
                                    qT[g][:, h * 512 + lo:(h + 1) * 512],
                                    start=True, stop=True,
                                )
                                if j >= 0:
                                    a, b = ((0, 128), (128, 256),
                                            (256, 384), (384, 512))[j]
                                    nc.vector.tensor_add(
                                        ps_s[:, base + a:base + b],
                                        ps_s[:, base + a:base + b],
                                        mask_sb[:, j, a:b],
                                    )
                            ex = pexp.tile([128, 1024], BF16, tag="ex")
                            if los[1] == 0:
                                nc.scalar.activation(
                                    ex[:, los[0]:1024], ps_s[:, los[0]:1024],
                                    mybir.ActivationFunctionType.Exp,
                                    scale=0.125,
                                )
                            else:
                                nc.scalar.activation(
                                    ex[:, los[0]:512], ps_s[:, los[0]:512],
                                    mybir.ActivationFunctionType.Exp,
                                    scale=0.125,
                                )
                                nc.scalar.activation(
                                    ex[:, 512 + los[1]:1024],
                                    ps_s[:, 512 + los[1]:1024],
                                    mybir.ActivationFunctionType.Exp,
                                    scale=0.125,
                                )
                            if pend_av and len(pend_av) > 1:
                                pend_av.pop(0)()
                            if pend_norm is not None and pk >= 1:
                                pend_norm()
                                pend_norm = None

                            def make_av(kcs=kcs, los=tuple(los), ex=ex,
                                        ps_av=ps_av):
                                def emit():
                                    for idx, kc in enumerate(kcs):
                                        kg, ko = divmod(kc, 4)
                                        lo = los[idx]
                                        base = idx * 512
                                        nc.tensor.matmul(
                                            ps_av[0:65, lo:512],
                                            vo[kg][:, ko, :],
                                            ex[:, base + lo:base + 512],
                                            start=(kc == 0),
                                            stop=(kc == nch - 1),
                                        )
                                return emit

                            pend_av.append(make_av())
                            ci += 2
                            if ci > lead:
                                pull((4, 3, 2, 1)[g])
                        for av in pend_av:
                            av()
                        if pend_norm is not None:
                            pend_norm()
                        pend_norm = make_norm(ps_av, p0, hp2)
                    pend_norm()

                # ---------------- phase 3: output projection ----------------
                def phase3_tile(t):
                    g, tt = divmod(t, 4)
                    o_sb = po.tile([128, E], BF16, tag="o")
                    for eb in range(4):
                        # last group's out-proj runs in the rep tail: keep it
                        # off the qkv ring so the next rep's phase 1 starts.
                        # mid-rep out-proj alternates rings to halve ring
                        # pressure (copy latency gates every 2nd alloc).
                        if g == ng - 1:
                            ps_o = psV.tile([128, 512], F32, tag="v",
                                            name="ps_o")[:]
                        else:
                            ps_o = psA.tile([128, 512], F32, tag="a",
                                            name="ps_o")[:]
                        for c in range(2):
                            nc.tensor.matmul(
                                ps_o,
                                r(ao[g][:, c, tt * 128:(tt + 1) * 128]),
                                r(woT_sb[:, c, eb * 512:(eb + 1) * 512]),
                                start=(c == 0), stop=(c == 1),
                            )
                        if eb < 2:
                            nc.scalar.copy(o_sb[:, eb * 512:(eb + 1) * 512], ps_o)
                        else:
                            nc.vector.tensor_copy(
                                o_sb[:, eb * 512:(eb + 1) * 512], ps_o
                            )
                        yield
                    eng = nc.gpsimd if tt % 2 == 0 else nc.sync
                    eng.dma_start(
                        out=out_d.ap()[t * 128:(t + 1) * 128, :], in_=o_sb[:]
                    )
                    yield

                def phase3_group(g):
                    for tt in range(4):
                        yield from phase3_tile(4 * g + tt)

                # Software pipeline: a deque of pending fill generators feeds
                # PE work (next group's phase 1, previous groups' phase 3)
                # into the Act-paced attention chunk stream.
                from collections import deque

                fills = deque()

                def pull_one():
                    while fills:
                        if next(fills[0], StopIteration) is StopIteration:
                            fills.popleft()
                        else:
                            return True
                    return False

                def drain_all():
                    while pull_one():
                        pass

                def drain_gen(gen):
                    # FIFO-pull until `gen` has been fully emitted
                    while gen in fills:
                        if not pull_one():
                            break

                p1gens = {}
                if 1 in phases:
                    for _ in phase1_group(0):
                        pass
                for g in range(ng):
                    if 1 in phases and g + 1 < ng:
                        p1gens[g + 1] = phase1_group(g + 1)
                        fills.append(p1gens[g + 1])
                    # delay phase-3 fills so they land in the later, bigger
                    # (Act-paced) attention groups: P3(g) feeds P2(g+2)
                    if 3 in phases and g >= 2 and (g - 2) not in (3,):
                        fills.append(phase3_group(g - 2))
                    if g == ng - 1 and 3 in phases:
                        fills.append(phase3_group(g - 1))
                    if 2 in phases:
                        if g in p1gens:
                            drain_gen(p1gens[g])
                        phase2_group(g, fill=pull_one,
                                     lead=(4 if g == 0 else 0))
                    else:
                        drain_all()
                if 3 in phases:
                    fills.append(phase3_group(ng - 1))
                drain_all()

    nc.compile()
    return nc


def make_tables(s_n=S):
    """Host-side RoPE tables and the S^T-layout additive causal masks."""
    theta = (1.0 / (10000.0 ** (np.arange(0, HD, 2, dtype=np.float32) / HD))).astype(
        np.float32
    )
    freqs = np.arange(s_n, dtype=np.float32)[:, None] * theta[None, :]  # [s, 32]
    cos = np.cos(freqs).astype(np.float32)
    sin = np.sin(freqs).astype(np.float32)
    cosh = np.tile(cos, (1, DQK // HD))  # [s, 160]
    sinh = np.tile(sin, (1, DQK // HD))
    cs = np.ascontiguousarray(
        np.concatenate([cosh, sinh], axis=1)
    )  # [s, 320] = [cos | sin]
    # maskt[r, j*512 + c] = 0 iff q-col c >= 128*j + k-row r (unmasked)
    r_ = np.arange(128)[:, None]
    c_ = np.arange(512)[None, :]
    blocks = [
        np.where(c_ >= 128 * j + r_, 0.0, MASK_NEG).astype(np.float32)
        for j in range(4)
    ]
    maskt = np.ascontiguousarray(np.concatenate(blocks, axis=1))  # [128, 2048]
    return cs, maskt


def make_core_inputs(x2, wq, wk, wv, wo, core):
    """Per-core input dict (host-side sharding prep)."""
    cs, maskt = _TABLES
    i = core
    wq_i = wq[i * DQ:(i + 1) * DQ]
    wk_i = wk[i * DKV:(i + 1) * DKV]
    wv_i = wv[i * DKV:(i + 1) * DKV]
    wt = np.ascontiguousarray(np.concatenate([wq_i, wk_i, wv_i], axis=0).T)
    wot = np.ascontiguousarray(wo[:, i * DQ:(i + 1) * DQ].T)
    return {
        "xt": np.ascontiguousarray(np.asarray(x2, np.float32).T).astype(NP_BF16),
        "wt": wt.astype(NP_BF16),
        "wot": wot.astype(np.float32),
        "cs": cs,
        "maskt": maskt,
        "ident": np.eye(128, dtype=np.float32),
        "identb": np.eye(128, dtype=np.float32).astype(NP_BF16),
        "ones": np.ones((128, 64), dtype=np.float32),
        "onesb": np.ones((128, 4), dtype=np.float32).astype(NP_BF16),
    }


_TABLES = make_tables()
_NC_CACHE = {}


def _get_nc(reps=1):
    key = ("nc", reps)
    if key not in _NC_CACHE:
        _NC_CACHE[key] = build_nc(reps=reps)
    return _NC_CACHE[key]


def kernel(x, wq, wk, wv, wo):
    x = np.asarray(x, dtype=np.float32)
    b, s_n, e = x.shape
    x2 = np.ascontiguousarray(x.reshape(s_n, e))
    in_maps = [
        make_core_inputs(x2, np.asarray(wq, np.float32), np.asarray(wk, np.float32),
                         np.asarray(wv, np.float32), np.asarray(wo, np.float32), i)
        for i in range(NCORES)
    ]
    res = run_bass_kernel_spmd(_get_nc(), in_maps, core_ids=list(range(NCORES)))
    out = np.zeros((s_n, e), dtype=np.float32)
    for rr in res.results:
        out += np.asarray(rr["out"]).astype(np.float32)
    return out.reshape(b, s_n, e).astype(np.float32)
